# revision 4
# baseline (speedup 1.0000x reference)
"""AdaptiveGaussianConvLayer Trainium2 kernel (8 NeuronCores, SPMD, no collectives).

Math: out[b, j, d] = sum_i V[b, i, d] * W[b, i, j],
      W[b, i, j] = exp(-0.5 * ((j - i - mu[b,i]) / sigma[b,i])^2)
with B=4, N=4096, D=512; sigma in (0.5, 2.5), mu ~ 3*N(0,1).

W underflows to exactly 0.0 in fp32 once |j - i - mu|/sigma >= ~13.2, i.e. for
|j - i| >= ~48.  Each 128-wide j-tile therefore only needs the 3 aligned
128-row i-slabs centered on it (band 384 covers |j-i| <= 128); the rest is
exactly zero, so the banded result matches the dense reference to fp32
rounding.

Sharding: 8 cores = (batch b) x (j-half h).  Core c computes
out[b, h*2048:(h+1)*2048, :].  Host pads V/sigma/mu with 128 zero rows each
side of the core's i-window so all cores run one identical SPMD program
(pad rows have V=0 and contribute nothing).

Per-core dataflow (i on partitions, j/d on the free axis):
  z_s   = (iota + q_s) * r_s            per slab s   (GpSimd/DVE tensor_scalar)
  z2    = Square(z)                     2-slab chunks (ACT, const scale/bias)
  W     = Exp(-0.5 * z2)                2-slab chunks (ACT), dtype bf16/f32r
  Vc    = cast(V)                       3-slab chunks (GpSimd/DVE)
  psum_t = sum_k W[slab t+k, cols] .T @ Vc[slab t+k]   (TensorE, k=0..2)
  obuf  <- psum (DVE/ACT copy), DMA out in 2-j-tile pairs
where q_s[p] = -128 - p - mu_s[p], r_s[p] = 1/sigma_s[p], iota = row 0..383;
slab s covers local j in [128(s-2), 128(s+2)) so z = (j - i - mu)/sigma.
"""

import os
import numpy as np

import concourse.bass as bass
import concourse.bacc as bacc
import concourse.mybir as mybir
import concourse.tile as tile
from concourse.bass_utils import run_bass_kernel_spmd

AF = mybir.ActivationFunctionType
ALU = mybir.AluOpType

B, N, D = 4, 4096, 512
NCORES = 8
HALF = N // 2            # 2048 j per core
NSLAB = HALF // 128 + 2  # 18 i-slabs of 128 rows (1 pad slab each side)
VROWS = NSLAB * 128      # 2304
JT = HALF // 128         # 16 j-tiles per core
WWIN = 384               # j-window width per slab
WCH = 2                  # slabs per Square/Exp chunk
NWCH = NSLAB // WCH      # 9 chunks
VCH = 3                  # slabs per cast chunk
NVCH = NSLAB // VCH      # 6 cast chunks

DT_MM = os.environ.get("AGC_DT_MM", "f32r")

_cached = {}


def build_nc(dt_mm: str = DT_MM):
    f32 = mybir.dt.float32
    mm_dt = {"bf16": mybir.dt.bfloat16, "f32r": mybir.dt.float32r}[dt_mm]
    nc = bacc.Bacc("TRN2", target_bir_lowering=False, debug=False)

    vp_d = nc.dram_tensor("Vp", [VROWS, D], f32, kind="ExternalInput").ap()
    sb_d = nc.dram_tensor("sb", [128, 2 * NSLAB + 1], f32, kind="ExternalInput").ap()
    iota_d = nc.dram_tensor("iota", [128, WWIN], f32, kind="ExternalInput").ap()
    out_d = nc.dram_tensor("out", [HALF, D], f32, kind="ExternalOutput").ap()

    with tile.TileContext(nc) as tc:
        with (
            tc.tile_pool(name="const", bufs=1) as constp,
            tc.tile_pool(name="big", bufs=1) as bigp,
            tc.tile_pool(name="ps", bufs=4, space=bass.MemorySpace.PSUM) as pspool,
            tc.tile_pool(name="obuf", bufs=3) as opool,
        ):
            iota_t = constp.tile([128, WWIN], f32)
            sb_t = constp.tile([128, 2 * NSLAB + 1], f32)
            nc.sync.dma_start(iota_t[:], iota_d[:])
            nc.sync.dma_start(sb_t[:], sb_d[:])
            zero = sb_t[:, 2 * NSLAB : 2 * NSLAB + 1]

            vbuf = bigp.tile([128, NSLAB, D], f32)      # raw V slabs
            vc = bigp.tile([128, NSLAB, D], mm_dt)      # cast V slabs
            zbuf = bigp.tile([128, NSLAB, WWIN], f32)   # z per slab
            z2buf = bigp.tile([128, NSLAB, WWIN], f32)  # z^2
            wbuf = bigp.tile([128, NSLAB, WWIN], mm_dt)  # W

            # V loads: 3 big DMAs of 6 slabs each (1.5 MB -> full DMA BW)
            vp3 = vp_d.rearrange("(s p) d -> p s d", p=128)
            for c in range(3):
                nc.sync.dma_start(vbuf[:, 6 * c : 6 * (c + 1), :],
                                  vp3[:, 6 * c : 6 * (c + 1), :])

            # cast chunks: alternate GpSimd / DVE
            def emit_cast(c):
                dst = vc[:, VCH * c : VCH * (c + 1), :]
                src = vbuf[:, VCH * c : VCH * (c + 1), :]
                eng = nc.gpsimd if c % 2 == 0 else nc.vector
                eng.tensor_copy(dst, src)

            # z per slab on GpSimd (only needs iota + sb)
            def emit_z(s):
                nc.gpsimd.tensor_scalar(
                    zbuf[:, s, :], iota_t[:],
                    sb_t[:, 2 * s : 2 * s + 1],      # q  (add)
                    sb_t[:, 2 * s + 1 : 2 * s + 2],  # r  (mult)
                    ALU.add, ALU.mult,
                )

            # Square + Exp on ACT, 2-slab chunks (const scale/bias -> mergeable)
            def emit_sq_exp(c):
                zin = zbuf[:, WCH * c : WCH * (c + 1), :]
                z2 = z2buf[:, WCH * c : WCH * (c + 1), :]
                w = wbuf[:, WCH * c : WCH * (c + 1), :]
                nc.scalar.activation(z2, zin, AF.Square, bias=zero)
                nc.scalar.activation(w, z2, AF.Exp, bias=zero, scale=-0.5)

            # matmuls for one j-tile + copy to obuf half
            out3 = out_d.rearrange("(P h p) d -> P p h d", h=2, p=128)

            def emit_jtile(t, ob):
                ps = pspool.tile([128, D], f32)
                for k in range(3):
                    ls = t + k
                    nc.tensor.matmul(
                        ps[:],
                        wbuf[:, ls, (2 - k) * 128 : (3 - k) * 128],
                        vc[:, ls, :],
                        start=(k == 0),
                        stop=(k == 2),
                    )
                eng = nc.vector if t % 4 != 3 else nc.scalar
                if eng is nc.scalar:
                    nc.scalar.activation(ob[:, t % 2, :], ps[:], AF.Copy)
                else:
                    nc.vector.tensor_copy(ob[:, t % 2, :], ps[:])

            # pipeline emission: z -> chunks -> j-tiles as they become ready
            emit_z(0)
            emit_z(1)
            next_t = 0
            ob = None
            for c in range(NWCH):
                if 2 * c + 2 < NSLAB:
                    emit_z(2 * c + 2)
                if 2 * c + 3 < NSLAB:
                    emit_z(2 * c + 3)
                if c < NVCH:
                    emit_cast(c)
                emit_sq_exp(c)
                # j-tile t needs slabs t..t+2 (W) and cast chunk of those slabs
                max_slab_ready = 2 * c + 1
                max_cast_ready = VCH * min(c + 1, NVCH) - 1
                while next_t < JT and next_t + 2 <= min(max_slab_ready, max_cast_ready):
                    t = next_t
                    if t % 2 == 0:
                        ob = opool.tile([128, 2, D], f32)
                    emit_jtile(t, ob)
                    if t % 2 == 1:
                        nc.sync.dma_start(out3[t // 2], ob[:])
                    next_t += 1
            while next_t < JT:
                t = next_t
                if t % 2 == 0:
                    ob = opool.tile([128, 2, D], f32)
                emit_jtile(t, ob)
                if t % 2 == 1:
                    nc.sync.dma_start(out3[t // 2], ob[:])
                next_t += 1

    nc.compile()
    return nc


def _get_nc():
    if DT_MM not in _cached:
        _cached[DT_MM] = build_nc(DT_MM)
    return _cached[DT_MM]


def make_in_maps(V, sigma, mu):
    """Host-side sharding: per-core padded V rows + scale table."""
    V = np.asarray(V, dtype=np.float32)
    sigma = np.asarray(sigma, dtype=np.float32).reshape(B, N)
    mu = np.asarray(mu, dtype=np.float32).reshape(B, N)
    iota_arr = np.ascontiguousarray(
        np.broadcast_to(np.arange(WWIN, dtype=np.float32), (128, WWIN))
    )
    pidx = (np.arange(VROWS) % 128).astype(np.float32)
    in_maps = []
    for c in range(NCORES):
        b, h = divmod(c, 2)
        jb = h * HALF
        lo, hi = jb - 128, jb + HALF + 128
        slo, shi = max(lo, 0), min(hi, N)
        vp = np.zeros((VROWS, D), np.float32)
        sig = np.ones(VROWS, np.float32)
        muv = np.zeros(VROWS, np.float32)
        vp[slo - lo : shi - lo] = V[b, slo:shi]
        sig[slo - lo : shi - lo] = sigma[b, slo:shi]
        muv[slo - lo : shi - lo] = mu[b, slo:shi]
        r = (np.float32(1.0) / sig).astype(np.float32)
        q = (np.float32(-128.0) - pidx - muv).astype(np.float32)
        sb = np.zeros((128, 2 * NSLAB + 1), np.float32)
        sb[:, 0:2 * NSLAB:2] = q.reshape(NSLAB, 128).T
        sb[:, 1:2 * NSLAB:2] = r.reshape(NSLAB, 128).T
        in_maps.append({"Vp": vp, "sb": sb, "iota": iota_arr})
    return in_maps


def gather(results):
    out = np.empty((B, N, D), np.float32)
    for c in range(NCORES):
        b, h = divmod(c, 2)
        out[b, h * HALF : (h + 1) * HALF] = np.asarray(results[c]["out"])
    return out


def kernel(V, sigma, mu):
    nc = _get_nc()
    in_maps = make_in_maps(V, sigma, mu)
    res = run_bass_kernel_spmd(nc, in_maps, core_ids=list(range(NCORES)))
    return gather(res.results)


# revision 6
# speedup vs baseline: 1.3348x; 1.3348x over previous
"""AdaptiveGaussianConvLayer Trainium2 kernel (8 NeuronCores, SPMD, no collectives).

Math: out[b, j, d] = sum_i V[b, i, d] * W[b, i, j],
      W[b, i, j] = exp(-0.5 * ((j - i - mu[b,i]) / sigma[b,i])^2)
with B=4, N=4096, D=512; sigma in (0.5, 2.5), mu ~ 3*N(0,1).

W underflows to exactly 0.0 in fp32 once |j - i - mu|/sigma >= ~13.2, i.e. for
|j - i| >= ~48.  Each 128-wide j-tile therefore only needs the 3 aligned
128-row i-slabs centered on it (band 384 covers |j-i| <= 128); the rest is
exactly zero, so the banded result matches the dense reference to fp32
rounding.

Sharding: 8 cores = (batch b) x (j-half h).  Core c computes
out[b, h*2048:(h+1)*2048, :].  Host pads V/sigma/mu with 128 zero rows each
side of the core's i-window so all cores run one identical SPMD program
(pad rows have V=0 and contribute nothing).

Per-core dataflow (i on partitions, j/d on the free axis):
  z_s   = (iota + q_s) * r_s            per slab s   (GpSimd/DVE tensor_scalar)
  z2    = Square(z)                     2-slab chunks (ACT, const scale/bias)
  W     = Exp(-0.5 * z2)                2-slab chunks (ACT), dtype bf16/f32r
  Vc    = cast(V)                       3-slab chunks (GpSimd/DVE)
  psum_t = sum_k W[slab t+k, cols] .T @ Vc[slab t+k]   (TensorE, k=0..2)
  obuf  <- psum (DVE/ACT copy), DMA out in 2-j-tile pairs
where q_s[p] = -128 - p - mu_s[p], r_s[p] = 1/sigma_s[p], iota = row 0..383;
slab s covers local j in [128(s-2), 128(s+2)) so z = (j - i - mu)/sigma.
"""

import os
import numpy as np

import concourse.bass as bass
import concourse.bacc as bacc
import concourse.mybir as mybir
import concourse.tile as tile
from concourse.bass_utils import run_bass_kernel_spmd

AF = mybir.ActivationFunctionType
ALU = mybir.AluOpType

B, N, D = 4, 4096, 512
NCORES = 8
HALF = N // 2            # 2048 j per core
NSLAB = HALF // 128 + 2  # 18 i-slabs of 128 rows (1 pad slab each side)
VROWS = NSLAB * 128      # 2304
JT = HALF // 128         # 16 j-tiles per core
WWIN = 384               # j-window width per slab
WCH = 3                  # slabs per Square/Exp/cast chunk
NWCH = NSLAB // WCH      # 6 chunks

DT_MM = os.environ.get("AGC_DT_MM", "bf16")

_cached = {}


def build_nc(dt_mm: str = DT_MM):
    f32 = mybir.dt.float32
    mm_dt = {"bf16": mybir.dt.bfloat16, "f32r": mybir.dt.float32r}[dt_mm]
    nc = bacc.Bacc("TRN2", target_bir_lowering=False, debug=False)

    vp_d = nc.dram_tensor("Vp", [VROWS, D], f32, kind="ExternalInput").ap()
    sb_d = nc.dram_tensor("sb", [128, 2 * NSLAB + 1], f32, kind="ExternalInput").ap()
    iota_d = nc.dram_tensor("iota", [128, WWIN], f32, kind="ExternalInput").ap()
    out_d = nc.dram_tensor("out", [HALF, D], f32, kind="ExternalOutput").ap()

    with tile.TileContext(nc) as tc:
        with (
            tc.tile_pool(name="const", bufs=1) as constp,
            tc.tile_pool(name="big", bufs=1) as bigp,
            tc.tile_pool(name="ps", bufs=3, space=bass.MemorySpace.PSUM) as pspool,
            tc.tile_pool(name="obuf", bufs=3) as opool,
        ):
            iota_t = constp.tile([128, WWIN], f32)
            sb_t = constp.tile([128, 2 * NSLAB + 1], f32)
            nc.sync.dma_start(iota_t[:], iota_d[:])
            nc.sync.dma_start(sb_t[:], sb_d[:])
            zero = sb_t[:, 2 * NSLAB : 2 * NSLAB + 1]

            # flat 2D tiles (3D APs defeat DVE fast modes)
            vbuf = bigp.tile([128, NSLAB * D], f32)      # raw V slabs
            vc = bigp.tile([128, NSLAB * D], mm_dt)      # cast V slabs
            zbuf = bigp.tile([128, NSLAB * WWIN], f32)   # z per slab
            z2buf = bigp.tile([128, NSLAB * WWIN], f32)  # z^2
            wbuf = bigp.tile([128, NSLAB * WWIN], mm_dt)  # W

            # V loads: 3 big DMAs of 6 slabs each (1.5 MB -> full DMA BW),
            # issued on the ACT HWDGE ring to keep the SP ring free for outputs
            vp3 = vp_d.rearrange("(s p) d -> p s d", p=128)
            vb3 = vbuf[:].rearrange("p (s d) -> p s d", d=D)
            for c in range(3):
                nc.scalar.dma_start(vb3[:, 6 * c : 6 * (c + 1), :],
                                    vp3[:, 6 * c : 6 * (c + 1), :])

            # cast chunk on DVE (2D contiguous slice)
            def emit_cast(c):
                dst = vc[:, WCH * c * D : WCH * (c + 1) * D]
                src = vbuf[:, WCH * c * D : WCH * (c + 1) * D]
                nc.vector.tensor_copy(dst, src)

            # z per slab: first 6 on DVE (fast, gates ACT start), rest GpSimd
            def emit_z(s):
                eng = nc.vector if s < 6 else nc.gpsimd
                eng.tensor_scalar(
                    zbuf[:, s * WWIN : (s + 1) * WWIN], iota_t[:],
                    sb_t[:, 2 * s : 2 * s + 1],      # q  (add)
                    sb_t[:, 2 * s + 1 : 2 * s + 2],  # r  (mult)
                    ALU.add, ALU.mult,
                )

            # Square + Exp on ACT, 3-slab chunks (const scale/bias -> mergeable)
            def emit_sq_exp(c):
                lo, hi = WCH * c * WWIN, WCH * (c + 1) * WWIN
                nc.scalar.activation(z2buf[:, lo:hi], zbuf[:, lo:hi],
                                     AF.Square, bias=zero)
                nc.scalar.activation(wbuf[:, lo:hi], z2buf[:, lo:hi],
                                     AF.Exp, bias=zero, scale=-0.5)

            out3 = out_d.rearrange("(P h p) d -> P p h d", h=2, p=128)

            # matmuls for one j-tile into half of a paired PSUM tile
            def emit_jtile(t, ps):
                half = t % 2
                for k in range(3):
                    ls = t + k
                    nc.tensor.matmul(
                        ps[:, half * D : (half + 1) * D],
                        wbuf[:, ls * WWIN + (2 - k) * 128 : ls * WWIN + (3 - k) * 128],
                        vc[:, ls * D : (ls + 1) * D],
                        start=(k == 0),
                        stop=(k == 2),
                    )

            def emit_pair_out(p, ps):
                ob = opool.tile([128, 2 * D], f32)
                if p % 4 == 3:
                    nc.scalar.activation(ob[:], ps[:], AF.Copy)
                else:
                    nc.vector.tensor_copy(ob[:], ps[:])
                nc.sync.dma_start(
                    out3[p], ob[:].rearrange("p (h d) -> p h d", h=2))

            # pipeline emission
            for s in range(3):
                emit_z(s)
            next_t = 0
            ps = None
            for c in range(NWCH):
                for s in range(WCH * c + 3, min(WCH * (c + 1) + 3, NSLAB)):
                    emit_z(s)
                emit_cast(c)
                emit_sq_exp(c)
                # j-tile t needs W+cast of slabs t..t+2 -> t <= 3c
                while next_t < JT and next_t + 2 <= WCH * (c + 1) - 1:
                    t = next_t
                    if t % 2 == 0:
                        ps = pspool.tile([128, 2 * D], f32)
                    emit_jtile(t, ps)
                    if t % 2 == 1:
                        emit_pair_out(t // 2, ps)
                    next_t += 1
            while next_t < JT:
                t = next_t
                if t % 2 == 0:
                    ps = pspool.tile([128, 2 * D], f32)
                emit_jtile(t, ps)
                if t % 2 == 1:
                    emit_pair_out(t // 2, ps)
                next_t += 1

    nc.compile()
    return nc


def _get_nc():
    if DT_MM not in _cached:
        _cached[DT_MM] = build_nc(DT_MM)
    return _cached[DT_MM]


def make_in_maps(V, sigma, mu):
    """Host-side sharding: per-core padded V rows + scale table."""
    V = np.asarray(V, dtype=np.float32)
    sigma = np.asarray(sigma, dtype=np.float32).reshape(B, N)
    mu = np.asarray(mu, dtype=np.float32).reshape(B, N)
    iota_arr = np.ascontiguousarray(
        np.broadcast_to(np.arange(WWIN, dtype=np.float32), (128, WWIN))
    )
    pidx = (np.arange(VROWS) % 128).astype(np.float32)
    in_maps = []
    for c in range(NCORES):
        b, h = divmod(c, 2)
        jb = h * HALF
        lo, hi = jb - 128, jb + HALF + 128
        slo, shi = max(lo, 0), min(hi, N)
        vp = np.zeros((VROWS, D), np.float32)
        sig = np.ones(VROWS, np.float32)
        muv = np.zeros(VROWS, np.float32)
        vp[slo - lo : shi - lo] = V[b, slo:shi]
        sig[slo - lo : shi - lo] = sigma[b, slo:shi]
        muv[slo - lo : shi - lo] = mu[b, slo:shi]
        r = (np.float32(1.0) / sig).astype(np.float32)
        q = (np.float32(-128.0) - pidx - muv).astype(np.float32)
        sb = np.zeros((128, 2 * NSLAB + 1), np.float32)
        sb[:, 0:2 * NSLAB:2] = q.reshape(NSLAB, 128).T
        sb[:, 1:2 * NSLAB:2] = r.reshape(NSLAB, 128).T
        in_maps.append({"Vp": vp, "sb": sb, "iota": iota_arr})
    return in_maps


def gather(results):
    out = np.empty((B, N, D), np.float32)
    for c in range(NCORES):
        b, h = divmod(c, 2)
        out[b, h * HALF : (h + 1) * HALF] = np.asarray(results[c]["out"])
    return out


def kernel(V, sigma, mu):
    nc = _get_nc()
    in_maps = make_in_maps(V, sigma, mu)
    res = run_bass_kernel_spmd(nc, in_maps, core_ids=list(range(NCORES)))
    return gather(res.results)


# revision 7
# speedup vs baseline: 1.3987x; 1.0479x over previous
"""AdaptiveGaussianConvLayer Trainium2 kernel (8 NeuronCores, SPMD, no collectives).

Math: out[b, j, d] = sum_i V[b, i, d] * W[b, i, j],
      W[b, i, j] = exp(-0.5 * ((j - i - mu[b,i]) / sigma[b,i])^2)
with B=4, N=4096, D=512; sigma in (0.5, 2.5), mu ~ 3*N(0,1).

W underflows to exactly 0.0 in fp32 once |j - i - mu|/sigma >= ~13.2, i.e. for
|j - i| >= ~48.  Each 128-wide j-tile therefore only needs the 3 aligned
128-row i-slabs centered on it (band 384 covers |j-i| <= 128); the rest is
exactly zero, so the banded result matches the dense reference to fp32
rounding.

Sharding: 8 cores = (batch b) x (j-half h).  Core c computes
out[b, h*2048:(h+1)*2048, :].  Host pads V/sigma/mu with 128 zero rows each
side of the core's i-window so all cores run one identical SPMD program
(pad rows have V=0 and contribute nothing).  V is pre-cast to bf16 on the
host (the matmul compute dtype), halving its DMA traffic.

Per-core dataflow (i on partitions, j/d on the free axis):
  z2_s  = Square(r_s * iota + b0_s)     per slab s  (ACT, per-partition scale
                                         r=1/sigma, bias b0=(-128-p-mu)*r)
  W     = Exp(-0.5 * z2)                multi-slab chunks (ACT, bf16 out)
  psum  = sum_k W[slab t+k] .T @ V[slab t+k]   (TensorE, bf16, k=0..2)
  obuf  <- psum copy (DVE/ACT), DMA out in 2-j-tile pairs
Slab s covers local j in [128(s-2), 128(s+2)); with u the iota value,
z = (u - 128 - p - mu)/sigma = (j - i - mu)/sigma.  A few slabs' z2 are
computed on GpSimd (tensor_scalar + square) to offload the ScalarE chain.
"""

import os
import numpy as np
import ml_dtypes

import concourse.bass as bass
import concourse.bacc as bacc
import concourse.mybir as mybir
import concourse.tile as tile
from concourse.bass_utils import run_bass_kernel_spmd

AF = mybir.ActivationFunctionType
ALU = mybir.AluOpType

B, N, D = 4, 4096, 512
NCORES = 8
HALF = N // 2            # 2048 j per core
NSLAB = HALF // 128 + 2  # 18 i-slabs of 128 rows (1 pad slab each side)
VROWS = NSLAB * 128      # 2304
JT = HALF // 128         # 16 j-tiles per core
WWIN = 384               # j-window width per slab

# (start_col, width) of the genuinely used j-window per slab: edge slabs
# serve fewer j-tiles.  Slab s serves j-tiles t in {s-2, s-1, s} & [0, JT).
def _slab_win(s):
    t_lo, t_hi = max(s - 2, 0), min(s, JT - 1)
    lo = (t_lo - (s - 2)) * 128
    return lo, (t_hi - t_lo + 1) * 128

# Exp/unlock chunks: slabs grouped [0-2],[3-5],[6-8],[9-11],[12-13],[14-15],[16-17]
CHUNKS = [(0, 3), (3, 3), (6, 3), (9, 3), (12, 2), (14, 2), (16, 2)]

# slabs whose z2 is computed on GpSimd instead of ACT (offload)
GPS_SLABS = frozenset((4, 7, 10, 13))

DT_MM = "bf16"

_cached = {}


def build_nc():
    f32 = mybir.dt.float32
    bf16 = mybir.dt.bfloat16
    nc = bacc.Bacc("TRN2", target_bir_lowering=False, debug=False)

    # V is pre-cast to bf16 on the host
    vp_d = nc.dram_tensor("Vp", [VROWS, D], bf16, kind="ExternalInput").ap()
    # cst = [iota(384) | q r pairs (36) | b0 r pairs (36) | zero] per partition
    CW = WWIN + 4 * NSLAB + 1
    cst_d = nc.dram_tensor("cst", [128, CW], f32, kind="ExternalInput").ap()
    out_d = nc.dram_tensor("out", [HALF, D], f32, kind="ExternalOutput").ap()

    with tile.TileContext(nc) as tc:
        with (
            tc.tile_pool(name="const", bufs=1) as constp,
            tc.tile_pool(name="big", bufs=1) as bigp,
            tc.tile_pool(name="ps", bufs=3, space=bass.MemorySpace.PSUM) as pspool,
            tc.tile_pool(name="obuf", bufs=3) as opool,
        ):
            cst_t = constp.tile([128, CW], f32)
            nc.sync.dma_start(cst_t[:], cst_d[:])
            iota_t = cst_t[:, 0:WWIN]
            qr = lambda s: (cst_t[:, WWIN + 2 * s : WWIN + 2 * s + 1],
                            cst_t[:, WWIN + 2 * s + 1 : WWIN + 2 * s + 2])
            b0r = lambda s: (cst_t[:, WWIN + 2 * NSLAB + 2 * s : WWIN + 2 * NSLAB + 2 * s + 1],
                             cst_t[:, WWIN + 2 * NSLAB + 2 * s + 1 : WWIN + 2 * NSLAB + 2 * s + 2])
            zero = cst_t[:, CW - 1 : CW]

            vbuf = bigp.tile([128, NSLAB * D], bf16)      # V slabs (bf16)
            zbuf = bigp.tile([128, NSLAB * WWIN], f32)    # z (GpSimd slabs only)
            z2buf = bigp.tile([128, NSLAB * WWIN], f32)   # z^2
            wbuf = bigp.tile([128, NSLAB * WWIN], bf16)   # W

            # V loads: 1.5-slab-chunk DMAs split over both HWDGE rings
            vp3 = vp_d.rearrange("(s p) d -> p s d", p=128)
            vb3 = vbuf[:].rearrange("p (s d) -> p s d", d=D)
            for c in range(3):
                eng = nc.sync if c == 0 else nc.scalar
                eng.dma_start(vb3[:, 6 * c : 6 * (c + 1), :],
                              vp3[:, 6 * c : 6 * (c + 1), :])

            # z2 per slab: ACT Square(r*iota + b0), or GpSimd z=(iota+q)*r, z*z
            def emit_z2(s):
                lo, w = _slab_win(s)
                z2 = z2buf[:, s * WWIN + lo : s * WWIN + lo + w]
                src = iota_t[:, lo : lo + w]
                if s in GPS_SLABS:
                    q, r = qr(s)
                    z = zbuf[:, s * WWIN + lo : s * WWIN + lo + w]
                    nc.gpsimd.tensor_scalar(z, src, q, r, ALU.add, ALU.mult)
                    nc.gpsimd.tensor_tensor(z2, z, z, ALU.mult)
                else:
                    b0, r = b0r(s)
                    nc.scalar.activation(z2, src, AF.Square, bias=b0, scale=r)

            # Exp chunk -> W (const scale/bias, mergeable across slabs)
            def emit_exp(ci):
                s0, ns = CHUNKS[ci]
                lo = s0 * WWIN + _slab_win(s0)[0]
                last = s0 + ns - 1
                hi = last * WWIN + sum(_slab_win(last))
                nc.scalar.activation(wbuf[:, lo:hi], z2buf[:, lo:hi],
                                     AF.Exp, bias=zero, scale=-0.5)

            out3 = out_d.rearrange("(P h p) d -> P p h d", h=2, p=128)

            def emit_jtile(t, ps):
                half = t % 2
                for k in range(3):
                    ls = t + k
                    nc.tensor.matmul(
                        ps[:, half * D : (half + 1) * D],
                        wbuf[:, ls * WWIN + (2 - k) * 128 : ls * WWIN + (3 - k) * 128],
                        vbuf[:, ls * D : (ls + 1) * D],
                        start=(k == 0),
                        stop=(k == 2),
                    )

            def emit_pair_out(p, ps):
                ob = opool.tile([128, 2 * D], f32)
                if p % 4 == 1:
                    nc.scalar.activation(ob[:], ps[:], AF.Copy)
                else:
                    nc.vector.tensor_copy(ob[:], ps[:])
                nc.sync.dma_start(
                    out3[p], ob[:].rearrange("p (h d) -> p h d", h=2))

            # pipeline emission: z2 -> Exp chunks -> j-tiles as they unlock
            next_t = 0
            ps = None
            for ci, (s0, ns) in enumerate(CHUNKS):
                for s in range(s0, s0 + ns):
                    emit_z2(s)
                emit_exp(ci)
                # j-tile t needs W of slabs t..t+2  ->  t <= s0+ns-3
                while next_t < JT and next_t <= s0 + ns - 3:
                    t = next_t
                    if t % 2 == 0:
                        ps = pspool.tile([128, 2 * D], f32)
                    emit_jtile(t, ps)
                    if t % 2 == 1:
                        emit_pair_out(t // 2, ps)
                    next_t += 1
            assert next_t == JT

    nc.compile()
    return nc


def _get_nc():
    if "nc" not in _cached:
        _cached["nc"] = build_nc()
    return _cached["nc"]


def make_in_maps(V, sigma, mu):
    """Host-side sharding: per-core padded bf16 V rows + scale table."""
    V = np.asarray(V, dtype=np.float32)
    sigma = np.asarray(sigma, dtype=np.float32).reshape(B, N)
    mu = np.asarray(mu, dtype=np.float32).reshape(B, N)
    CW = WWIN + 4 * NSLAB + 1
    pidx = (np.arange(VROWS) % 128).astype(np.float32)
    in_maps = []
    for c in range(NCORES):
        b, h = divmod(c, 2)
        jb = h * HALF
        lo, hi = jb - 128, jb + HALF + 128
        slo, shi = max(lo, 0), min(hi, N)
        vp = np.zeros((VROWS, D), ml_dtypes.bfloat16)
        sig = np.ones(VROWS, np.float32)
        muv = np.zeros(VROWS, np.float32)
        vp[slo - lo : shi - lo] = V[b, slo:shi].astype(ml_dtypes.bfloat16)
        sig[slo - lo : shi - lo] = sigma[b, slo:shi]
        muv[slo - lo : shi - lo] = mu[b, slo:shi]
        r = (np.float32(1.0) / sig).astype(np.float32)
        q = (np.float32(-128.0) - pidx - muv).astype(np.float32)
        b0 = (q * r).astype(np.float32)
        cst = np.zeros((128, CW), np.float32)
        cst[:, 0:WWIN] = np.arange(WWIN, dtype=np.float32)[None, :]
        cst[:, WWIN : WWIN + 2 * NSLAB : 2] = q.reshape(NSLAB, 128).T
        cst[:, WWIN + 1 : WWIN + 2 * NSLAB : 2] = r.reshape(NSLAB, 128).T
        cst[:, WWIN + 2 * NSLAB : WWIN + 4 * NSLAB : 2] = b0.reshape(NSLAB, 128).T
        cst[:, WWIN + 2 * NSLAB + 1 : WWIN + 4 * NSLAB : 2] = r.reshape(NSLAB, 128).T
        in_maps.append({"Vp": vp, "cst": cst})
    return in_maps


def gather(results):
    out = np.empty((B, N, D), np.float32)
    for c in range(NCORES):
        b, h = divmod(c, 2)
        out[b, h * HALF : (h + 1) * HALF] = np.asarray(results[c]["out"])
    return out


def kernel(V, sigma, mu):
    nc = _get_nc()
    in_maps = make_in_maps(V, sigma, mu)
    res = run_bass_kernel_spmd(nc, in_maps, core_ids=list(range(NCORES)))
    return gather(res.results)


# revision 11
# speedup vs baseline: 1.4941x; 1.0682x over previous
"""AdaptiveGaussianConvLayer Trainium2 kernel (8 NeuronCores, SPMD, no collectives).

Math: out[b, j, d] = sum_i V[b, i, d] * W[b, i, j],
      W[b, i, j] = exp(-0.5 * ((j - i - mu[b,i]) / sigma[b,i])^2)
with B=4, N=4096, D=512; sigma in (0.5, 2.5), mu ~ 3*N(0,1).

W underflows to exactly 0.0 in fp32 once |j - i - mu|/sigma >= ~13.2, i.e. for
|j - i| >= ~48.  Each 128-wide j-tile therefore only needs the 3 aligned
128-row i-slabs centered on it (band 384 covers |j-i| <= 128); the rest is
exactly zero, so the banded result matches the dense reference to fp32
rounding.

Sharding: 8 cores = (batch b) x (j-half h).  Core c computes
out[b, h*2048:(h+1)*2048, :].  Host pads V/sigma/mu with 128 zero rows each
side of the core's i-window so all cores run one identical SPMD program
(pad rows have V=0 and contribute nothing).  V is pre-cast to bf16 on the
host (the matmul compute dtype), halving its DMA traffic.

Per-core dataflow (i on partitions, j/d on the free axis):
  z2_s  = Square(r_s * iota + b0_s)     per slab s  (ACT, per-partition scale
                                         r=1/sigma, bias b0=(-128-p-mu)*r)
  W     = Exp(-0.5 * z2)                multi-slab chunks (ACT, bf16 out)
  psum  = sum_k W[slab t+k] .T @ V[slab t+k]   (TensorE, bf16, k=0..2)
  obuf  <- psum copy (DVE/ACT), DMA out in 2-j-tile pairs
Slab s covers local j in [128(s-2), 128(s+2)); with u the iota value,
z = (u - 128 - p - mu)/sigma = (j - i - mu)/sigma.  A few slabs' z2 are
computed on GpSimd (tensor_scalar + square) to offload the ScalarE chain.
"""

import os
import numpy as np
import ml_dtypes

import concourse.bass as bass
import concourse.bacc as bacc
import concourse.mybir as mybir
import concourse.tile as tile
from concourse.bass_utils import run_bass_kernel_spmd

AF = mybir.ActivationFunctionType
ALU = mybir.AluOpType

B, N, D = 4, 4096, 512
NCORES = 8
HALF = N // 2            # 2048 j per core
NSLAB = HALF // 128 + 2  # 18 i-slabs of 128 rows (1 pad slab each side)
VROWS = NSLAB * 128      # 2304
JT = HALF // 128         # 16 j-tiles per core
WWIN = 384               # j-window width per slab

# (start_col, width) of the genuinely used j-window per slab: edge slabs
# serve fewer j-tiles.  Slab s serves j-tiles t in {s-2, s-1, s} & [0, JT).
def _slab_win(s):
    t_lo, t_hi = max(s - 2, 0), min(s, JT - 1)
    lo = (t_lo - (s - 2)) * 128
    return lo, (t_hi - t_lo + 1) * 128

# Exp/unlock chunks; small final chunks shorten the post-ACT tail
CHUNKS = [(0, 3), (3, 3), (6, 3), (9, 3), (12, 3), (15, 1), (16, 1), (17, 1)]

# slabs whose z2 is computed on GpSimd instead of ACT (offload); these are
# emitted first so GpSimd runs ahead of the ACT chain rather than stalling it
GPS_SLABS = (3, 6, 9, 12, 14, 16)

DT_MM = "bf16"

_cached = {}


def _lean_drain_and_barrier(self, tick_clock, wait_clock):
    """Tail trimmed: drop the second all-engine barrier after the semaphore
    clears.  nrt_execute only returns when every engine's stream is done, and
    the next execution starts fresh, so the clears (last instructions on
    GpSimd) cannot race a re-execution."""
    from concourse.vector_clock import ScopedClock

    drain_inst = self.nc.sync.drain()
    wait_clock.add_sem_waits(
        drain_inst.ins, ScopedClock({None: tick_clock.global_clock})
    )
    self.nc.all_engine_barrier()
    popped = self.nc._tile_sem_poison_stack.pop()
    assert popped is self._sem_poison
    self.nc.clear_and_free_semaphores(list(self.sems.allocated().values()))


def build_nc():
    tile.TileContext._drain_and_barrier = _lean_drain_and_barrier
    f32 = mybir.dt.float32
    bf16 = mybir.dt.bfloat16
    nc = bacc.Bacc("TRN2", target_bir_lowering=False, debug=False)

    # V is pre-cast to bf16 on the host
    vp_d = nc.dram_tensor("Vp", [VROWS, D], bf16, kind="ExternalInput").ap()
    # cst = [iota(384) | q r pairs (36) | b0 r pairs (36) | zero] per partition
    CW = WWIN + 4 * NSLAB + 1
    cst_d = nc.dram_tensor("cst", [128, CW], f32, kind="ExternalInput").ap()
    out_d = nc.dram_tensor("out", [HALF, D], f32, kind="ExternalOutput").ap()

    with tile.TileContext(nc) as tc:
        with (
            tc.tile_pool(name="const", bufs=1) as constp,
            tc.tile_pool(name="big", bufs=1) as bigp,
            tc.tile_pool(name="ps", bufs=3, space=bass.MemorySpace.PSUM) as pspool,
            tc.tile_pool(name="obuf", bufs=3) as opool,
        ):
            cst_t = constp.tile([128, CW], f32)
            nc.sync.dma_start(cst_t[:], cst_d[:])
            iota_t = cst_t[:, 0:WWIN]
            qr = lambda s: (cst_t[:, WWIN + 2 * s : WWIN + 2 * s + 1],
                            cst_t[:, WWIN + 2 * s + 1 : WWIN + 2 * s + 2])
            b0r = lambda s: (cst_t[:, WWIN + 2 * NSLAB + 2 * s : WWIN + 2 * NSLAB + 2 * s + 1],
                             cst_t[:, WWIN + 2 * NSLAB + 2 * s + 1 : WWIN + 2 * NSLAB + 2 * s + 2])
            zero = cst_t[:, CW - 1 : CW]

            vbuf = bigp.tile([128, NSLAB * D], bf16)      # V slabs (bf16)
            zbuf = bigp.tile([128, NSLAB * WWIN], f32)    # z (GpSimd slabs only)
            z2buf = bigp.tile([128, NSLAB * WWIN], f32)   # z^2
            wbuf = bigp.tile([128, NSLAB * WWIN], bf16)   # W

            # V loads on the ACT HWDGE ring (cst + outputs own the SP ring);
            # the issue cost overlaps ACT's wait for cst anyway
            vp3 = vp_d.rearrange("(s p) d -> p s d", p=128)
            vb3 = vbuf[:].rearrange("p (s d) -> p s d", d=D)
            for c in range(3):
                nc.scalar.dma_start(vb3[:, 6 * c : 6 * (c + 1), :],
                                    vp3[:, 6 * c : 6 * (c + 1), :])

            # z2 per slab: ACT Square(r*iota + b0), or GpSimd z=(iota+q)*r, z*z
            def emit_z2(s):
                lo, w = _slab_win(s)
                z2 = z2buf[:, s * WWIN + lo : s * WWIN + lo + w]
                src = iota_t[:, lo : lo + w]
                if s in GPS_SLABS:
                    q, r = qr(s)
                    z = zbuf[:, s * WWIN + lo : s * WWIN + lo + w]
                    nc.gpsimd.tensor_scalar(z, src, q, r, ALU.add, ALU.mult)
                    nc.gpsimd.tensor_tensor(z2, z, z, ALU.mult)
                else:
                    b0, r = b0r(s)
                    nc.scalar.activation(z2, src, AF.Square, bias=b0, scale=r)

            # Exp chunk -> W (const scale/bias, mergeable across slabs)
            def emit_exp(ci):
                s0, ns = CHUNKS[ci]
                lo = s0 * WWIN + _slab_win(s0)[0]
                last = s0 + ns - 1
                hi = last * WWIN + sum(_slab_win(last))
                nc.scalar.activation(wbuf[:, lo:hi], z2buf[:, lo:hi],
                                     AF.Exp, bias=zero, scale=-0.5)

            out3 = out_d.rearrange("(P h p) d -> P p h d", h=2, p=128)

            def emit_jtile(t, ps):
                half = t % 2
                for k in range(3):
                    ls = t + k
                    nc.tensor.matmul(
                        ps[:, half * D : (half + 1) * D],
                        wbuf[:, ls * WWIN + (2 - k) * 128 : ls * WWIN + (3 - k) * 128],
                        vbuf[:, ls * D : (ls + 1) * D],
                        start=(k == 0),
                        stop=(k == 2),
                    )

            # pipeline emission: GpSimd z2 first (runs ahead), then per-chunk
            # ACT z2 -> Exp -> j-tiles as they unlock
            for s in GPS_SLABS:
                emit_z2(s)
            next_t = 0
            ps = ob = None
            for ci, (s0, ns) in enumerate(CHUNKS):
                for s in range(s0, s0 + ns):
                    if s not in GPS_SLABS:
                        emit_z2(s)
                emit_exp(ci)
                # j-tile t needs W of slabs t..t+2  ->  t <= s0+ns-3
                while next_t < JT and next_t <= s0 + ns - 3:
                    t = next_t
                    if t % 2 == 0:
                        ps = pspool.tile([128, 2 * D], f32)
                        ob = opool.tile([128, 2 * D], f32)
                    emit_jtile(t, ps)
                    nc.vector.tensor_copy(ob[:, (t % 2) * D : (t % 2 + 1) * D],
                                          ps[:, (t % 2) * D : (t % 2 + 1) * D])
                    if t % 2 == 1:
                        nc.sync.dma_start(
                            out3[t // 2], ob[:].rearrange("p (h d) -> p h d", h=2))
                    next_t += 1
            assert next_t == JT

    nc.compile()
    return nc


def _get_nc():
    if "nc" not in _cached:
        _cached["nc"] = build_nc()
    return _cached["nc"]


def make_in_maps(V, sigma, mu):
    """Host-side sharding: per-core padded bf16 V rows + scale table."""
    V = np.asarray(V, dtype=np.float32)
    sigma = np.asarray(sigma, dtype=np.float32).reshape(B, N)
    mu = np.asarray(mu, dtype=np.float32).reshape(B, N)
    CW = WWIN + 4 * NSLAB + 1
    pidx = (np.arange(VROWS) % 128).astype(np.float32)
    in_maps = []
    for c in range(NCORES):
        b, h = divmod(c, 2)
        jb = h * HALF
        lo, hi = jb - 128, jb + HALF + 128
        slo, shi = max(lo, 0), min(hi, N)
        vp = np.zeros((VROWS, D), ml_dtypes.bfloat16)
        sig = np.ones(VROWS, np.float32)
        muv = np.zeros(VROWS, np.float32)
        vp[slo - lo : shi - lo] = V[b, slo:shi].astype(ml_dtypes.bfloat16)
        sig[slo - lo : shi - lo] = sigma[b, slo:shi]
        muv[slo - lo : shi - lo] = mu[b, slo:shi]
        r = (np.float32(1.0) / sig).astype(np.float32)
        q = (np.float32(-128.0) - pidx - muv).astype(np.float32)
        b0 = (q * r).astype(np.float32)
        cst = np.zeros((128, CW), np.float32)
        cst[:, 0:WWIN] = np.arange(WWIN, dtype=np.float32)[None, :]
        cst[:, WWIN : WWIN + 2 * NSLAB : 2] = q.reshape(NSLAB, 128).T
        cst[:, WWIN + 1 : WWIN + 2 * NSLAB : 2] = r.reshape(NSLAB, 128).T
        cst[:, WWIN + 2 * NSLAB : WWIN + 4 * NSLAB : 2] = b0.reshape(NSLAB, 128).T
        cst[:, WWIN + 2 * NSLAB + 1 : WWIN + 4 * NSLAB : 2] = r.reshape(NSLAB, 128).T
        in_maps.append({"Vp": vp, "cst": cst})
    return in_maps


def gather(results):
    out = np.empty((B, N, D), np.float32)
    for c in range(NCORES):
        b, h = divmod(c, 2)
        out[b, h * HALF : (h + 1) * HALF] = np.asarray(results[c]["out"])
    return out


def kernel(V, sigma, mu):
    nc = _get_nc()
    in_maps = make_in_maps(V, sigma, mu)
    res = run_bass_kernel_spmd(nc, in_maps, core_ids=list(range(NCORES)))
    return gather(res.results)
